# revision 4
# baseline (speedup 1.0000x reference)
"""Correntropy loss on 8 Trainium2 NeuronCores — centered-fp8 staging,
PE-subtract + ACT/DVE squares.

Reference math (all f32):
    t = (target - 0.5) * 2 ; o = (output - 0.5) * 2
    cost = mean(1 - exp(-sigma * (o - t)^2)),  sigma = 1/1000
Since o - t == 2*(output - target):
    cost = mean(1 - exp(-c * w)),  w = (output - target)^2,  c = 4*sigma

The kernel is HBM-bandwidth-bound; the rel-err budget (2e-2) is far
above f32 staging needs, so the host stages both tensors as CENTERED
fp8-e4m3 (q = fp8(x - 0.5)): 1/4 the bytes of f32.  Centering halves
e4m3's ulp over the data range.  Measured on the real key-0 data:
centered fp8 + 1-term series -> rel err 1.9e-3 (gate is 2e-2).

Device per core (row shard 8192 x 1000, folded to [128, 64000] cols):
    d = qo - qt  exactly, then S1 = sum(d^2), via two parallel routes:
  * PE route (~2/3 of cols): one DoubleRow fp8 matmul per 512-col chunk
    with stationary [I | -I] computes d into PSUM f32 exactly (2 rows/
    cyc, ~0.42 ns/col, weight reloads hidden).  ACT consumes 2048-col
    PSUM groups (4 banks) with Square + f32 accumulator (~1.1 ns/col);
    two groups ping-pong across the 8 PSUM banks.
  * DVE route (~1/3): tensor_sub fp8,fp8->bf16 (1 cyc/col) then
    scalar_tensor_tensor d*d with f32 accum (1 cyc/col).
Host reduces the partial-sum columns in f64 and applies cost ~= c*S1/N
(dropping the -c^2/2*S2 term: 8e-4 relative, inside the budget).

Schedule: 9 DMA tiles.  Tile 0 is small (2048 cols, pure PE route) so
compute starts ~2 us in; a dummy 1-col activation at t=0 prefetches the
Square LUT (1.3 us) under the first DMA.  The last tile is smaller and
its DVE work column-sliced to keep the post-final-DMA chain short.
DMA: 16.4 MB/core ~ 45.6 us at the ~360 GB/s per-core HBM roofline;
ACT ~47 us and DVE ~47 us run just at/above it.
"""

import numpy as np
import ml_dtypes

import concourse.bacc as bacc
import concourse.mybir as mybir
import concourse.tile as tile
from concourse.bass_utils import run_bass_kernel_spmd

N_CORES = 8
ROWS = 65536
COLS = 1000
ROWS_PER_CORE = ROWS // N_CORES  # 8192
P = 128  # SBUF partitions
TOTAL = ROWS_PER_CORE * COLS // P  # 64000 cols per operand per partition

GW = 2048  # ACT consumes PSUM in groups of GW cols (4 banks)
CW = 512  # one matmul / PSUM bank worth of cols

# Per DMA tile: (free_cols, dve_cols, n_pe_groups, n_dve_slices)
# with dve_cols + GW*n_pe_groups == free_cols; sum(free) == TOTAL.
TILE_CFG = [
    (2048, 0, 1, 0),
    (8000, 1856, 3, 1),
    (8000, 1856, 3, 1),
    (8000, 1856, 3, 1),
    (8000, 1856, 3, 1),
    (8000, 3904, 2, 1),
    (8000, 3904, 2, 1),
    (8000, 3904, 2, 2),
    (5952, 1856, 2, 2),
]
N_TILES = len(TILE_CFG)
assert sum(c[0] for c in TILE_CFG) == TOTAL
assert all(dw + GW * ng == fr for fr, dw, ng, _ in TILE_CFG)
TILE_OFF = np.cumsum([0] + [c[0] for c in TILE_CFG]).tolist()  # col offsets

DVE_PIECES = []  # (tile, col_off, width)
for _t, (_fr, _dw, _g, _ns) in enumerate(TILE_CFG):
    for _k in range(_ns):
        step = _dw // _ns
        DVE_PIECES.append((_t, _k * step, step if _k < _ns - 1 else _dw - _k * step))
N_DVE = len(DVE_PIECES)
N_GRP = sum(c[2] for c in TILE_CFG)  # 21
ACC_COLS = N_DVE + N_GRP

F32 = mybir.dt.float32
BF16 = mybir.dt.bfloat16
FP8 = mybir.dt.float8e4


def _build():
    nc = bacc.Bacc()
    comb_p = nc.declare_dram_parameter(
        "combined", [2 * P, TOTAL], FP8, isOutput=False
    )
    wid_p = nc.declare_dram_parameter("wid", [P, 2 * P], FP8, isOutput=False)
    acc_p = nc.declare_dram_parameter("partial", [P, ACC_COLS], F32, isOutput=True)

    # [2, P, TOTAL] -> [P, 2, TOTAL] view; tiles slice the last axis
    comb_v = comb_p[:].rearrange("(c p) m -> p c m", c=2, p=P)

    with tile.TileContext(nc) as tc:
        with (
            tc.tile_pool(name="io", bufs=1) as io_pool,
            tc.tile_pool(name="work", bufs=1) as work_pool,
            tc.tile_pool(name="accp", bufs=1) as acc_pool,
            tc.tile_pool(name="ps", bufs=1, space="PSUM") as ps_pool,
        ):
            acc = acc_pool.tile([P, ACC_COLS], F32)
            stat = acc_pool.tile([P, 2 * P], FP8)
            nc.sync.dma_start(out=stat[:], in_=wid_p[:])
            stat_v = stat[:].rearrange("p (c m) -> p c m", c=2)

            Sq = mybir.ActivationFunctionType.Square
            M = mybir.AluOpType.mult

            # dummy activation at t=0: prefetches the Square table set
            warm = acc_pool.tile([P, 1], F32)
            nc.vector.memset(warm[:], 0.0)
            nc.scalar.activation(warm[:], warm[:], Sq)

            ab_tiles = {}

            def get_ab(t):
                if t not in ab_tiles:
                    fr = TILE_CFG[t][0]
                    ab = io_pool.tile([P, 2 * fr], FP8, tag=f"ab{fr}",
                                      bufs=(4 if fr == 8000 else 1))
                    o0 = TILE_OFF[t]
                    nc.sync.dma_start(
                        out=ab[:].rearrange("p (c m) -> p c m", c=2),
                        in_=comb_v[:, :, o0 : o0 + fr],
                    )
                    ab_tiles[t] = ab
                return ab_tiles[t]

            grp = 0
            dve_i = 0
            for t, (fr, dw, ng, ns) in enumerate(TILE_CFG):
                ab = get_ab(t)
                ab_v = ab[:].rearrange("p (c m) -> p c m", c=2)

                # PE route: cols [dw, fr) in GW groups of 4 matmuls
                for g in range(ng):
                    base = dw + GW * g
                    pg = ps_pool.tile([P, GW], F32, tag=f"pg{grp % 2}")
                    for k in range(GW // CW):
                        off = base + CW * k
                        nc.tensor.matmul(
                            pg[:, CW * k : CW * (k + 1)],
                            stat_v,
                            ab_v[:, :, off : off + CW],
                            start=True, stop=True,
                            perf_mode=mybir.MatmulPerfMode.DoubleRow,
                        )
                    wg = work_pool.tile([P, GW], FP8, tag="wg", bufs=2)
                    nc.scalar.activation(
                        wg[:], pg[:], Sq,
                        accum_out=acc[:, N_DVE + grp : N_DVE + grp + 1],
                    )
                    grp += 1

                # DVE route: cols [0, dw)
                while dve_i < N_DVE and DVE_PIECES[dve_i][0] == t:
                    _, off, z = DVE_PIECES[dve_i]
                    d = work_pool.tile([P, z], BF16, tag="d", bufs=2)
                    nc.vector.tensor_sub(
                        d[:], ab[:, off : off + z],
                        ab[:, fr + off : fr + off + z],
                    )
                    w = work_pool.tile([P, z], BF16, tag="w", bufs=2)
                    nc.vector.scalar_tensor_tensor(
                        out=w[:], in0=d[:], scalar=1.0, in1=d[:],
                        op0=M, op1=M,
                        accum_out=acc[:, dve_i : dve_i + 1],
                    )
                    dve_i += 1

            nc.sync.dma_start(out=acc_p[:], in_=acc[:])
    nc.finalize()
    return nc


_NC = None


def _get_nc():
    global _NC
    if _NC is None:
        _NC = _build()
    return _NC


def _shard_inputs(output, target):
    output = np.asarray(output, dtype=np.float32)
    target = np.asarray(target, dtype=np.float32)
    qo = (output - np.float32(0.5)).astype(ml_dtypes.float8_e4m3)
    qt = (target - np.float32(0.5)).astype(ml_dtypes.float8_e4m3)

    idn = np.zeros((P, P), dtype=ml_dtypes.float8_e4m3)
    np.fill_diagonal(idn, 1.0)
    nidn = np.zeros((P, P), dtype=ml_dtypes.float8_e4m3)
    np.fill_diagonal(nidn, -1.0)
    wid = np.concatenate([idn, nidn], axis=1)  # [P, 2P]: I then -I

    in_maps = []
    for i in range(N_CORES):
        sl = slice(i * ROWS_PER_CORE, (i + 1) * ROWS_PER_CORE)
        # [8192, 1000] -> [P, TOTAL]: partition p takes rows 64p..64p+63
        o2 = qo[sl].reshape(P, TOTAL)
        t2 = qt[sl].reshape(P, TOTAL)
        comb = np.concatenate([o2[None], t2[None]], axis=0).reshape(2 * P, TOTAL)
        in_maps.append({"combined": comb, "wid": wid})
    return in_maps


def run_device(output, target, trace=False):
    """Returns (per-core partial sum arrays, BassKernelResults)."""
    in_maps = _shard_inputs(output, target)
    res = run_bass_kernel_spmd(_get_nc(), in_maps, list(range(N_CORES)), trace=trace)
    partials = [res.results[i]["partial"] for i in range(N_CORES)]
    return partials, res


def _reduce(partials):
    s1 = 0.0
    for p in partials:
        s1 += p.astype(np.float64).sum()
    c = 4.0 * float(np.float32(1.0 / COLS))  # match reference's f32 sigma
    n = float(ROWS) * float(COLS)
    return np.array(c * s1 / n, dtype=np.float32)


def kernel(output, target):
    partials, _ = run_device(output, target)
    return _reduce(partials)
